# revision 48
# baseline (speedup 1.0000x reference)
"""Trainium2 Bass kernel for nn_Discriminator (DGCNN-style discriminator).

Sharding: data-parallel over batch. 16 point clouds -> 8 NeuronCores x 2.
No collectives; the host splits inputs and concatenates outputs.

Algorithm restructuring (exact, since lrelu is monotone and the 1x1 conv is
linear in the edge feature [x_j - x_i ; x_i]):
    edge_conv(x)[:, i] = lrelu( max_{j in knn(i)} u[:, j] + w[:, i] )
        u = W[:, :d] @ x            (per-point, no k dimension)
        w = (W[:, d:] - W[:, :d]) @ x
which removes the B*N*k*2d*out edge matmul entirely.

knn: top-20 of each row of P = x_i . x_j - ||x_j||^2/2 (same order as
-||x_i - x_j||^2 per row). Per 128-row tile: 3 rounds of DVE Max8
(max / max_index / match_replace). Neighbor gather via gpsimd dma_gather
straight out of SBUF (u rows staged point-major in fp16); the 20-neighbour
max runs as an fp16 tensor-tensor max tree on the DVE (2x perf mode).
The W4 reduction streams its weights on the Act HWDGE queue so it overlaps
the second batch's convs instead of serializing at the end.
"""

import numpy as np

B, N, KNN, NCORES = 16, 1024, 20, 8
BPC = B // NCORES  # batches per core
CONV_D = [6, 64, 64, 128]
CONV_O = [64, 64, 128, 256]
NEG = -1.0e30
FSIZES = [64, 64, 128, 128, 128]

_CACHE = {}


def _build_nc():
    import concourse.bacc as bacc
    import concourse.mybir as mybir
    import concourse.tile as tile
    from concourse.bass import ds, ts

    f32 = mybir.dt.float32
    f16 = mybir.dt.float16
    u16 = mybir.dt.uint16
    i16 = mybir.dt.int16
    AF = mybir.ActivationFunctionType
    ALU = mybir.AluOpType
    AX = mybir.AxisListType.X

    nc = bacc.Bacc("TRN2", target_bir_lowering=False,
                   dynamic_dma_scratch_size=2**16)

    xt_d = nc.dram_tensor("xt", [BPC, 6, N], f32, kind="ExternalInput")
    y_d = nc.dram_tensor("y", [BPC, 16], f32, kind="ExternalInput")
    wstk_d = [
        nc.dram_tensor(f"wstk{c}", [CONV_D[c], 2 * CONV_O[c]], f32, kind="ExternalInput")
        for c in range(4)
    ]
    w4t_d = nc.dram_tensor("w4t", [512, 1024], f32, kind="ExternalInput")
    l0t_d = nc.dram_tensor("l0t", [1088, 512], f32, kind="ExternalInput")
    l1t_d = nc.dram_tensor("l1t", [512, 256], f32, kind="ExternalInput")
    l2t_d = nc.dram_tensor("l2t", [256, 1], f32, kind="ExternalInput")
    f0t_d = nc.dram_tensor("f0t", [16, 16], f32, kind="ExternalInput")
    f1t_d = nc.dram_tensor("f1t", [16, 64], f32, kind="ExternalInput")
    f0b_d = nc.dram_tensor("f0b", [16, 1], f32, kind="ExternalInput")
    f1b_d = nc.dram_tensor("f1b", [64, 1], f32, kind="ExternalInput")
    l2b_d = nc.dram_tensor("l2b", [1, 1], f32, kind="ExternalInput")
    out_d = nc.dram_tensor("out", [BPC, 1], f32, kind="ExternalOutput")

    ctx = {}

    def lrelu_into(dst, src, rows):
        lt = ctx["mp"].tile([128, N], f32, tag="ltmp", name="ltmp")
        nc.scalar.mul(lt[:rows], src[:rows], 0.2)
        nc.vector.tensor_tensor(dst[:rows], src[:rows], lt[:rows], op=ALU.max)

    def y_embed():
        c = ctx
        consts, gzp, ps1 = c["consts"], c["gzp"], c["ps1"]
        f0t_sb = consts.tile([16, 16], f32, tag="f0t")
        nc.sync.dma_start(f0t_sb, f0t_d[:, :])
        f1t_sb = consts.tile([16, 64], f32, tag="f1t")
        nc.sync.dma_start(f1t_sb, f1t_d[:, :])
        f0b_sb = consts.tile([16, 1], f32, tag="f0b")
        nc.sync.dma_start(f0b_sb, f0b_d[:, :])
        f1b_sb = consts.tile([64, 1], f32, tag="f1b")
        nc.sync.dma_start(f1b_sb, f1b_d[:, :])
        ysb = consts.tile([16, BPC], f32, tag="ysb")
        for b in range(BPC):
            nc.sync.dma_start(
                ysb[:, ds(b, 1)], y_d[ds(b, 1)].rearrange("one p -> p one")
            )
        for b in range(BPC):
            yp = ps1.tile([128, 1], f32, tag="ps1")
            nc.tensor.matmul(yp[:16], f0t_sb, ysb[:, ds(b, 1)],
                             start=True, stop=True)
            yepre = gzp.tile([16, 1], f32, tag="yepre")
            nc.scalar.activation(yepre, yp[:16], AF.Identity, bias=f0b_sb)
            ye0 = gzp.tile([16, 1], f32, tag="ye0")
            nc.scalar.mul(ye0, yepre, 0.2)
            nc.vector.tensor_tensor(ye0, yepre, ye0, op=ALU.max)
            yp2 = ps1.tile([128, 1], f32, tag="ps1")
            nc.tensor.matmul(yp2[:64], f1t_sb, ye0, start=True, stop=True)
            ye1pre = gzp.tile([64, 1], f32, tag="ye1pre")
            nc.scalar.activation(ye1pre, yp2[:64], AF.Identity, bias=f1b_sb)
            ye1t = gzp.tile([64, 1], f32, tag="ye1t")
            nc.scalar.mul(ye1t, ye1pre, 0.2)
            nc.vector.tensor_tensor(c["yel"][b], ye1pre, ye1t, op=ALU.max)

    def conv_norms(xin, d, on_dve=False):
        """-0.5*||x_j||^2 row and (for d<=126) augmented operand tiles.
        For the first conv the DVE is idle, so staging runs there instead of
        queueing behind the Act engine."""
        c = ctx
        sq = c["sqp"].tile([128, N], f32, tag="sq")
        if on_dve:
            nc.vector.tensor_tensor(sq[:d], xin, xin, op=ALU.mult)
        else:
            nc.scalar.activation(sq[:d], xin, AF.Square)
        nxp = c["ps2"].tile([1, 2, 512], f32, tag="ps2")
        for h in range(2):
            nc.tensor.matmul(nxp[:, h], c["neghalf"][:d],
                             sq[:d, ds(h * 512, 512)], start=True, stop=True)
        nxx = c["sqp"].tile([1, N], f32, tag="nxx")
        nc.scalar.copy(nxx, nxp.rearrange("p a b -> p (a b)"))
        auglhs = augrhs = None
        if d <= 126:
            auglhs = c["augp"].tile([128, N], f32, tag="auglhs")
            augrhs = c["augp"].tile([128, N], f32, tag="augrhs")
            cpy = nc.vector.tensor_copy if on_dve else nc.scalar.copy
            cpy(auglhs[:d], xin)
            nc.sync.dma_start(auglhs[ds(d, 1)], c["ones_n"][:, :])
            cpy(augrhs[:d], xin)
            nc.sync.dma_start(augrhs[ds(d, 1)], nxx[:, :])
        return nxx, auglhs, augrhs

    def conv_uw(xin, cv, d, o, o16):
        """u rows point-major fp16 (for the gather) + w channel-major fp32."""
        c = ctx
        uw = c["uwp"].tile([128, 8, o16], f16, tag="uw")
        if o < o16:
            nc.vector.memset(uw[:, :, ds(o, o16 - o)], 0.0)
        for mm in range(8):
            up = c["ps1"].tile([128, o], f32, tag="ps1")
            nc.tensor.matmul(up, xin[:, ts(mm, 128)], c["wstk"][cv][:, :o],
                             start=True, stop=True)
            nc.scalar.copy(uw[:, mm, :o], up)
        wcm = []
        for j2 in range(max(1, o // 128)):
            ow = min(128, o)
            wt = c["wcmp"].tile([128, N], f32, tag=f"wcm{j2}", name=f"wcm{j2}")
            for h in range(2):
                wp = c["ps1"].tile([128, 512], f32, tag="ps1")
                nc.tensor.matmul(wp[:ow], c["wstk"][cv][:, ds(o + j2 * 128, ow)],
                                 xin[:, ds(h * 512, 512)], start=True, stop=True)
                nc.scalar.copy(wt[:ow, ds(h * 512, 512)], wp[:ow])
            wcm.append(wt)
        return uw, wcm

    def conv_topk_chunk(xin, d, nxx, auglhs, augrhs, ixall, cc):
        """pairwise + top-24 indices for one 128-row chunk."""
        c = ctx
        if True:
            pp = c["ps2"].tile([128, 2, 512], f32, tag="ps2")
            for h in range(2):
                if d <= 126:
                    nc.tensor.matmul(pp[:, h], auglhs[:d + 1, ts(cc, 128)],
                                     augrhs[:d + 1, ds(h * 512, 512)],
                                     start=True, stop=True)
                else:
                    nc.tensor.matmul(pp[:, h], xin[:, ts(cc, 128)],
                                     xin[:, ds(h * 512, 512)],
                                     start=True, stop=False)
                    nc.tensor.matmul(pp[:, h], c["ones_row"],
                                     nxx[:, ds(h * 512, 512)],
                                     start=False, stop=True)
            pw = c["pwp"].tile([128, N], f32, tag="pw")
            nc.scalar.copy(pw, pp.rearrange("p a b -> p (a b)"))
            mx = c["mxp"].tile([128, 24], f32, tag="mx")
            for r in range(3):
                nc.vector.max(mx[:, ds(8 * r, 8)], pw)
                nc.vector.max_index(
                    ixall[:, cc, ds(8 * r, 8)], mx[:, ds(8 * r, 8)], pw
                )
                if r < 2:
                    nc.vector.match_replace(
                        pw, in_to_replace=mx[:, ds(8 * r, 8)],
                        in_values=pw, imm_value=NEG,
                    )

    def conv_stage_idx_half(ixall, idxw, half):
        """idx tile for the transposed gather: list position
        j = 16*(t + 20*qh) + 2560*blk + ql -> entry [j%16, j//16].
        Staged in block-halves so gathers start mid-top-k. One DMA per qh
        covers 4 blocks, then partition replication."""
        c = ctx
        h4 = ds(4 * half, 4)
        for qh in range(8):
            nc.sync.dma_start(
                idxw[0:16, h4, ds(20 * qh, 20)],
                ixall.bitcast(i16)[ds(16 * qh, 16), h4, :20],
            )
        for rep in range(1, 8):
            nc.sync.dma_start(idxw[ds(16 * rep, 16), h4], idxw[0:16, h4])

    def conv_gather_max_pr(uw, idxw, o16, mcm, blk0, NB):
        """gather u rows from SBUF (transpose mode); 20-neighbour max via an
        fp16 TT-max tree on the DVE (2x mode)."""
        c = ctx
        EC = o16 // 128
        uwflat = uw.rearrange("p a b -> p (a b)")
        if True:
            G = c["gp"].tile([128, EC, NB * 2560], f16, tag="G")
            nc.gpsimd.dma_gather(
                G, uwflat,
                idxw[:, ds(blk0, NB)].rearrange("p a b -> p (a b)"),
                NB * 2560, NB * 2560, o16,
                transpose=True, single_packet=False,
                sbuf_tokens_per_rank=128,
                sbuf_free_dim_per_rank=o16 * 2,
            )
            # G[:, c, 2560*bi + 16*(t + 20*qh) + ql]
            Gv = G.rearrange("p c (bi qh t ql) -> p (c bi qh) t ql",
                             bi=NB, qh=8, t=20)
            M = EC * NB * 8
            t1 = c["tp"].tile([128, M, 10, 16], f16, tag="t1")
            nc.vector.tensor_tensor(t1, Gv[:, :, 0:10], Gv[:, :, 10:20],
                                    op=ALU.max)
            t2 = c["tp"].tile([128, M, 5, 16], f16, tag="t2")
            nc.vector.tensor_tensor(t2, t1[:, :, 0:5], t1[:, :, 5:10],
                                    op=ALU.max)
            t3 = c["tp"].tile([128, M, 2, 16], f16, tag="t1")
            nc.vector.tensor_tensor(t3, t2[:, :, 0:2], t2[:, :, 2:4],
                                    op=ALU.max)
            t4 = c["tp"].tile([128, M, 1, 16], f16, tag="t4")
            nc.vector.tensor_tensor(t4, t3[:, :, 0:1], t3[:, :, 1:2],
                                    op=ALU.max)
            for ec in range(EC):
                for bi in range(NB):
                    nc.vector.tensor_tensor(
                        mcm[ec][:, ts(blk0 + bi, 128)].rearrange(
                            "p (qh ql) -> p qh () ql", qh=8),
                        t4[:, ds((ec * NB + bi) * 8, 8)],
                        t2[:, ds((ec * NB + bi) * 8, 8), 4:5],
                        op=ALU.max)

    def conv_batch(cv, b):
        c = ctx
        d, o = CONV_D[cv], CONV_O[cv]
        xin = c["xt0"][b][:, :] if cv == 0 else c["feat"][(b, cv - 1)][:, :]
        nxx, auglhs, augrhs = conv_norms(xin, d, on_dve=(cv == 0))
        o16 = max(o, 128)
        uw, wcm = conv_uw(xin, cv, d, o, o16)
        EC = o16 // 128
        NB = 2 if EC == 1 else 1  # pair-merge gathers when they fit in SBUF
        mcm = [c["mp"].tile([128, N], f32, tag=f"mcm{ec}", name=f"mcm{ec}")
               for ec in range(EC)]
        ixall = c["ixp"].tile([128, 8, 24], u16, tag="ix")
        idxw = c["idxwp"].tile([128, 8, 160], i16, tag="idxw")
        for cc in range(8):
            conv_topk_chunk(xin, d, nxx, auglhs, augrhs, ixall, cc)
        for qh in range(8):
            nc.sync.dma_start(
                idxw[0:16, :, ds(20 * qh, 20)],
                ixall.bitcast(i16)[ds(16 * qh, 16), :, :20],
            )
        for rep in range(1, 8):
            nc.sync.dma_start(idxw[ds(16 * rep, 16)], idxw[0:16])
        if NB == 2:
            plan = [(0, 1), (1, 1), (2, 1), (3, 1), (4, 2), (6, 2)]
        else:
            plan = [(blk, 1) for blk in range(8)]
        for blk0, nb in plan:
            conv_gather_max_pr(uw, idxw, o16, mcm, blk0, nb)
        # f = lrelu(m + w), channel-major
        for j2 in range(max(1, o // 128)):
            ow = min(128, o)
            if cv <= 1:
                dstf = c["feat"][(b, cv)]
            elif cv == 2:
                dstf = c["feat"][(b, 2)]
            else:
                dstf = c["feat"][(b, 3 + j2)]
            nc.vector.tensor_add(mcm[j2][:ow], mcm[j2][:ow], wcm[j2][:ow])
            lrelu_into(dstf, mcm[j2], ow)

    def w4_stage(b):
        """h = W4 @ cat ; g2 = max_n h. Weights streamed per column block on
        the Act HWDGE queue so this overlaps the other batch's convs."""
        c = ctx
        gq = c["gzp"].tile([128, 16], f32, tag="gq")
        for mt in range(8):
            w4c = c["w4cp"].tile([128, 5, 128], f32, tag="w4c")
            row0 = 0
            for k in range(5):
                rows = FSIZES[k]
                nc.scalar.dma_start(w4c[:rows, k],
                                    w4t_d[ds(row0, rows), ts(mt, 128)])
                row0 += rows
            hp = c["ps2"].tile([128, 2, 512], f32, tag="ps2")
            for h2 in range(2):
                for k in range(5):
                    rows = FSIZES[k]
                    nc.tensor.matmul(hp[:, h2], w4c[:rows, k],
                                     c["feat"][(b, k)][:, ds(h2 * 512, 512)],
                                     start=(k == 0), stop=(k == 4))
                nc.vector.tensor_reduce(gq[:, ds(2 * mt + h2, 1)], hp[:, h2],
                                        axis=AX, op=ALU.max)
        nc.vector.tensor_reduce(c["g2"][b],
                                gq.rearrange("p (mt h) -> p mt h", h=2),
                                axis=AX, op=ALU.max)

    def final_stage(finp, psf):
        c = ctx
        l0t_sb = finp.tile([128, 9, 512], f32, tag="l0t")
        for k in range(9):
            rows = 128 if k < 8 else 64
            nc.sync.dma_start(l0t_sb[:rows, k], l0t_d[ds(128 * k, rows)])
        l1t_sb = finp.tile([128, 4, 256], f32, tag="l1t")
        for k in range(4):
            nc.sync.dma_start(l1t_sb[:, k], l1t_d[ds(128 * k, 128)])
        l2t_sb = finp.tile([128, 2, 1], f32, tag="l2t")
        for k in range(2):
            nc.sync.dma_start(l2t_sb[:, k], l2t_d[ds(128 * k, 128)])
        l2b_sb = finp.tile([1, 1], f32, tag="l2b")
        nc.sync.dma_start(l2b_sb, l2b_d[:, :])
        res = finp.tile([1, BPC], f32, tag="res")

        for b in range(BPC):
            z0 = finp.tile([128, 9], f32, tag=f"z0{b}")
            gt = finp.tile([128, 8], f32, tag=f"gt{b}", name="gt")
            nc.scalar.mul(gt, c["g2"][b], 0.2)
            nc.vector.tensor_tensor(z0[:, 0:8], c["g2"][b], gt, op=ALU.max)
            nc.scalar.copy(z0[0:64, ds(8, 1)], c["yel"][b])

            z1p = psf.tile([128, 4], f32, tag="zp")
            for mt in range(4):
                for k in range(9):
                    rows = 128 if k < 8 else 64
                    nc.tensor.matmul(z1p[:, ds(mt, 1)],
                                     l0t_sb[:rows, k, ts(mt, 128)],
                                     z0[:rows, ds(k, 1)],
                                     start=(k == 0), stop=(k == 8))
            z1 = finp.tile([128, 4], f32, tag=f"z1{b}")
            nc.scalar.copy(z1, z1p)
            z1t = finp.tile([128, 4], f32, tag=f"z1t{b}", name="z1t")
            nc.scalar.mul(z1t, z1, 0.2)
            nc.vector.tensor_tensor(z1, z1, z1t, op=ALU.max)
            z2p = psf.tile([128, 2], f32, tag="zp")
            for mt in range(2):
                for k in range(4):
                    nc.tensor.matmul(z2p[:, ds(mt, 1)],
                                     l1t_sb[:, k, ts(mt, 128)],
                                     z1[:, ds(k, 1)],
                                     start=(k == 0), stop=(k == 3))
            z2 = finp.tile([128, 2], f32, tag=f"z2{b}")
            nc.scalar.copy(z2, z2p)
            z2t = finp.tile([128, 2], f32, tag=f"z2t{b}", name="z2t")
            nc.scalar.mul(z2t, z2, 0.2)
            nc.vector.tensor_tensor(z2, z2, z2t, op=ALU.max)
            zp = psf.tile([1, 1], f32, tag="zp")
            for k in range(2):
                nc.tensor.matmul(zp, l2t_sb[:, k], z2[:, ds(k, 1)],
                                 start=(k == 0), stop=(k == 1))
            nc.scalar.activation(res[:, ds(b, 1)], zp, AF.Identity,
                                 bias=l2b_sb)
        nc.sync.dma_start(out_d.rearrange("b one -> one b"), res)

    with tile.TileContext(nc) as tc:
        with (
            tc.tile_pool(name="consts", bufs=1) as consts,
            tc.tile_pool(name="feat", bufs=1) as featp,
            tc.tile_pool(name="psf", bufs=1, space="PSUM") as psf,
        ):
            ones_row = consts.tile([1, 128], f32, tag="ones")
            nc.vector.memset(ones_row, 1.0)
            ones_n = consts.tile([1, N], f32, tag="ones_n")
            nc.vector.memset(ones_n, 1.0)
            neghalf = consts.tile([128, 1], f32, tag="neghalf")
            nc.vector.memset(neghalf, -0.5)
            wstk_sb = []
            for cvi in range(4):
                t = consts.tile([CONV_D[cvi], 2 * CONV_O[cvi]], f32,
                                tag=f"wstk{cvi}", name=f"wstk{cvi}")
                nc.sync.dma_start(t, wstk_d[cvi][:, :])
                wstk_sb.append(t)

            feat = {}
            for b in range(BPC):
                for fi, rows in enumerate(FSIZES):
                    feat[(b, fi)] = featp.tile([rows, N], f32, tag=f"f{b}_{fi}",
                                               name=f"f{b}_{fi}")
            xt0 = {}
            g2 = {}
            yel = {}
            for b in range(BPC):
                xt0[b] = featp.tile([6, N], f32, tag=f"xt0_{b}", name=f"xt0_{b}")
                nc.sync.dma_start(xt0[b], xt_d[b])
                g2[b] = featp.tile([128, 8], f32, tag=f"g2_{b}", name=f"g2_{b}")
                yel[b] = featp.tile([64, 1], f32, tag=f"yel_{b}", name=f"yel_{b}")

            ctx.update(consts=consts, featp=featp, feat=feat, xt0=xt0, g2=g2,
                       yel=yel, ones_row=ones_row, ones_n=ones_n,
                       neghalf=neghalf, wstk=wstk_sb)

            with (
                tc.tile_pool(name="sq", bufs=1) as sqp,
                tc.tile_pool(name="aug", bufs=2) as augp,
                tc.tile_pool(name="pw", bufs=2) as pwp,
                tc.tile_pool(name="mx", bufs=4) as mxp,
                tc.tile_pool(name="ix", bufs=2) as ixp,
                tc.tile_pool(name="uw", bufs=2) as uwp,
                tc.tile_pool(name="wcm", bufs=1) as wcmp,
                tc.tile_pool(name="idxw", bufs=2) as idxwp,
                tc.tile_pool(name="G", bufs=2) as gp,
                tc.tile_pool(name="T", bufs=1) as tp,
                tc.tile_pool(name="m", bufs=1) as mp,
                tc.tile_pool(name="w4c", bufs=2) as w4cp,
                tc.tile_pool(name="gz", bufs=2) as gzp,
                tc.tile_pool(name="ps2", bufs=2, space="PSUM") as ps2,
                tc.tile_pool(name="ps1", bufs=3, space="PSUM") as ps1,
            ):
                ctx.update(sqp=sqp, augp=augp, pwp=pwp, mxp=mxp, ixp=ixp,
                           uwp=uwp, wcmp=wcmp, idxwp=idxwp, gp=gp, tp=tp,
                           mp=mp, w4cp=w4cp, gzp=gzp, ps2=ps2, ps1=ps1)
                y_embed()
                for cv in range(4):
                    for b in range(BPC):
                        conv_batch(cv, b)
                for b in range(BPC):
                    w4_stage(b)

            with tc.tile_pool(name="fin", bufs=1) as finp:
                final_stage(finp, psf)

    nc.compile()
    return nc


def _get_nc():
    if "nc" not in _CACHE:
        _CACHE["nc"] = _build_nc()
    return _CACHE["nc"]


def make_in_maps(x, y, W0, W1, W2, W3, W4, L0, L1, L2_w, L2_b, F0_w, F0_b, F1_w, F1_b):
    def f32c(a):
        return np.ascontiguousarray(np.asarray(a, dtype=np.float32))

    x, y = f32c(x), f32c(y)
    xt = np.ascontiguousarray(np.swapaxes(x, 1, 2))  # [B, 6, N]

    def stk(W, d):
        W = f32c(W)
        w1p, w2p = W[:, :d], W[:, d:]
        return np.ascontiguousarray(
            np.concatenate([w1p.T, (w2p - w1p).T], axis=1)
        )

    base = {
        "wstk0": stk(W0, 6),
        "wstk1": stk(W1, 64),
        "wstk2": stk(W2, 64),
        "wstk3": stk(W3, 128),
        "w4t": np.ascontiguousarray(f32c(W4).T),
        "l0t": np.ascontiguousarray(f32c(L0).T),
        "l1t": np.ascontiguousarray(f32c(L1).T),
        "l2t": np.ascontiguousarray(f32c(L2_w).T),
        "f0t": np.ascontiguousarray(f32c(F0_w).T),
        "f1t": np.ascontiguousarray(f32c(F1_w).T),
        "f0b": f32c(F0_b).reshape(16, 1),
        "f1b": f32c(F1_b).reshape(64, 1),
        "l2b": f32c(L2_b).reshape(1, 1),
    }
    return [
        {**base, "xt": xt[c * BPC:(c + 1) * BPC], "y": y[c * BPC:(c + 1) * BPC]}
        for c in range(NCORES)
    ]


def kernel(**inputs):
    from concourse.bass_utils import run_bass_kernel_spmd

    nc = _get_nc()
    in_maps = make_in_maps(**inputs)
    res = run_bass_kernel_spmd(nc, in_maps, core_ids=list(range(NCORES)))
    return np.concatenate([r["out"] for r in res.results], axis=0)


if __name__ == "__main__":
    nc = _build_nc()
    print("built + compiled OK")
